# revision 2
# baseline (speedup 1.0000x reference)
"""GCN graph classification on 8 Trainium2 NeuronCores (Bass/Tile) — v2.

Structure (vs v1): AllGather chunks == gather windows (32768 table rows),
group-major/window-inner aggregation with PSUM accumulation across windows,
layer-2 feature matmuls + AllGather interleaved into layer-1 aggregation,
pooling interleaved into layer-2 aggregation, fp8 selection masks, and
16-partition index upload replicated on device.
"""
import sys

sys.path.insert(0, "/opt/trn_rl_repo")

import numpy as np
import ml_dtypes

import concourse.bass as bass
import concourse.bacc as bacc
import concourse.mybir as mybir
import concourse.tile as tile
from concourse.bass_utils import run_bass_kernel_spmd

# problem constants
N = 100000
E = 1600000
G = 512
H = 128
C = 10
NC = 8
NB = 98                # blocks per core
S = NB * 128           # node slots per core = 12544
NPAD = NC * S          # table rows = 100352
WIN = 32768
NWIN = 4
GRP = 4
NGRP = (NB + GRP - 1) // GRP  # 25
# window w covers slots [W_SLOT0[w], +W_ROWS[w]) on each core and table rows
# [W_BASE[w], +8*W_ROWS[w]); AllGather chunk == window.
W_BLKS = [32, 32, 32, 2]
W_ROWS = [b * 128 for b in W_BLKS]          # [4096,4096,4096,256]
W_SLOT0 = [0, 4096, 8192, 12288]
W_BASE = [0, 32768, 65536, 98304]

F32 = mybir.dt.float32
BF16 = mybir.dt.bfloat16
FP8 = mybir.dt.float8e4
I16 = mybir.dt.int16
NP_BF16 = ml_dtypes.bfloat16


def _blocks_of_group(g):
    return range(g * GRP, min((g + 1) * GRP, NB))


def _groups_of_window(w):
    # groups fully contained in window w (blocks 32w..32w+31; window 3: 96,97)
    if w < 3:
        return range(8 * w, 8 * (w + 1))
    return range(24, 25)


def preprocess(edge_index, batch):
    edge_index = np.asarray(edge_index, dtype=np.int64)
    batch = np.asarray(batch, dtype=np.int64)

    loop = np.arange(N, dtype=np.int64)
    src = np.concatenate([edge_index[0], loop])
    dst = np.concatenate([edge_index[1], loop])
    EE = src.shape[0]

    deg = np.bincount(dst, minlength=N).astype(np.float64)
    dinv = np.where(deg > 0, 1.0 / np.sqrt(deg), 0.0)
    csum = np.bincount(dst, weights=dinv[src], minlength=N)
    a = (dinv * csum).astype(np.float32)
    dinv32 = dinv.astype(np.float32)

    # node -> (core, slot): snake deal by descending degree
    order = np.argsort(-deg, kind="stable")
    pos = np.arange(N)
    p16 = pos % 16
    core_r = np.where(p16 < 8, p16, 15 - p16)
    j_r = (pos // 16) * 2 + (p16 >= 8)
    core = np.empty(N, dtype=np.int64)
    jwc = np.empty(N, dtype=np.int64)
    core[order] = core_r
    jwc[order] = j_r
    pas = jwc // NB
    r = jwc % NB
    blk = np.where(pas % 2 == 0, r, NB - 1 - r)
    slot = blk * 128 + pas
    assert pas.max() < 128

    # table row: window-aligned layout
    w_of_slot = np.minimum(slot // 4096, 3)
    rows_w = np.array(W_ROWS)[w_of_slot]
    base_w = np.array(W_BASE)[w_of_slot]
    slot0_w = np.array(W_SLOT0)[w_of_slot]
    tr = base_w + core * rows_w + (slot - slot0_w)
    assert tr.min() >= 0 and tr.max() < NPAD

    # per-slot arrays [NC, 128, NB]
    dinv_sl = np.zeros((NC, S), dtype=np.float32)
    a_sl = np.zeros((NC, S), dtype=np.float32)
    batc_sl = np.full((NC, S), -1.0, dtype=np.float32)
    dinv_sl[core, slot] = dinv32
    a_sl[core, slot] = a
    batc_sl[core, slot] = batch.astype(np.float32)

    def to_pj(x):
        return np.ascontiguousarray(x.reshape(NC, NB, 128).transpose(0, 2, 1))

    dinv_pj = to_pj(dinv_sl)
    a_pj = to_pj(a_sl)
    batc_pj = to_pj(batc_sl)

    # edges
    ecore = core[dst]
    eslot = slot[dst]
    eJ = eslot // 128
    eP = (eslot % 128).astype(np.float32)
    etr = tr[src]
    eq = etr // WIN
    eidx = (etr - eq * WIN).astype(np.int16)

    key = (ecore * NB + eJ) * NWIN + eq
    cnt = np.bincount(key, minlength=NC * NB * NWIN).reshape(NC, NB, NWIN)
    cq = np.ceil(cnt.max(axis=0) / 128).astype(np.int64)  # [NB, NWIN]

    # stream layout: for g: for q: for J in group
    seg_tok0 = np.zeros((NB, NWIN), dtype=np.int64)
    gathers = []  # (g, q, tok0, ntok, [(sub_tok0, sub_ntok, queue), ...])
    qload = [0, 0, 0, 0]
    tok = 0
    for g in range(NGRP):
        for q in range(NWIN):
            t0 = tok
            for J in _blocks_of_group(g):
                seg_tok0[J, q] = tok
                tok += cq[J, q] * 128
            ntok = tok - t0
            if ntok == 0:
                continue
            # split into sub-gathers of <= 1792 tokens; greedy queue balance
            nsub = max(1, int(np.ceil(ntok / 1792)))
            bounds = [t0 + (ntok * i // nsub) // 128 * 128 for i in range(nsub)]
            bounds.append(t0 + ntok)
            subs = []
            for i in range(nsub):
                s0, s1 = bounds[i], bounds[i + 1]
                if s1 <= s0:
                    continue
                qu = int(np.argmin(qload))
                qload[qu] += s1 - s0
                subs.append((s0, s1 - s0, qu))
            gathers.append((g, q, t0, ntok, subs))
    TOK = tok
    assert TOK % 128 == 0

    ordk = np.argsort(key, kind="stable")
    skey = key[ordk]
    first = np.searchsorted(skey, skey)
    rank = np.arange(EE) - first
    p_stream = seg_tok0[eJ[ordk], eq[ordk]] + rank

    gidx = np.zeros((NC, TOK), dtype=np.int16)
    dloc = np.full((NC, TOK), -1.0, dtype=np.float32)
    gidx[ecore[ordk], p_stream] = eidx[ordk]
    dloc[ecore[ordk], p_stream] = eP[ordk]

    # device layouts: gidx uploaded as 16 partitions (replicated on device)
    g16 = np.ascontiguousarray(
        gidx.reshape(NC, TOK // 16, 16).transpose(0, 2, 1)
    )  # [NC, 16, TOK//16]
    dloc_dev = np.ascontiguousarray(
        dloc.reshape(NC, TOK // 128, 128).transpose(0, 2, 1)
    ).astype(NP_BF16)

    cntg = np.bincount(batch, minlength=G).astype(np.float32)
    invcnt = (1.0 / np.maximum(cntg, 1.0)).reshape(4, 128).T.copy()

    sched = {"cq": cq, "seg_tok0": seg_tok0, "gathers": gathers, "TOK": TOK}
    percore = {
        "gidx16": g16,
        "dloc": dloc_dev,
        "dinv_pj": dinv_pj,
        "a_pj": a_pj,
        "batc_pj": batc_pj,
    }
    return sched, percore, invcnt


def build_program(sched):
    cq = sched["cq"]
    seg_tok0 = sched["seg_tok0"]
    gathers = sched["gathers"]
    TOK = sched["TOK"]
    gather_by_gq = {(g, q): (t0, nt, subs) for (g, q, t0, nt, subs) in gathers}

    nc = bacc.Bacc(
        "TRN2",
        target_bir_lowering=False,
        debug=False,
        num_devices=NC,
        num_swdge_queues=4,
    )

    din = {}
    din["gidx16"] = nc.dram_tensor("gidx16", [16, TOK // 16], I16, kind="ExternalInput")
    din["dloc"] = nc.dram_tensor("dloc", [128, TOK // 128], BF16, kind="ExternalInput")
    din["dinv"] = nc.dram_tensor("dinv", [128, NB], F32, kind="ExternalInput")
    din["acol"] = nc.dram_tensor("acol", [128, NB], F32, kind="ExternalInput")
    din["batchf"] = nc.dram_tensor("batchf", [128, NB], F32, kind="ExternalInput")
    din["W1"] = nc.dram_tensor("W1", [H, H], F32, kind="ExternalInput")
    din["W2"] = nc.dram_tensor("W2", [H, H], F32, kind="ExternalInput")
    din["Wp"] = nc.dram_tensor("Wp", [H, C], F32, kind="ExternalInput")
    din["W0r"] = nc.dram_tensor("W0r", [128, H], F32, kind="ExternalInput")
    din["b0r"] = nc.dram_tensor("b0r", [128, H], F32, kind="ExternalInput")
    din["b1r"] = nc.dram_tensor("b1r", [128, H], F32, kind="ExternalInput")
    din["b2r"] = nc.dram_tensor("b2r", [128, H], F32, kind="ExternalInput")
    din["bpr"] = nc.dram_tensor("bpr", [128, C], F32, kind="ExternalInput")
    din["ident"] = nc.dram_tensor("ident", [128, 128], F32, kind="ExternalInput")
    din["iotar"] = nc.dram_tensor("iotar", [128, 128], BF16, kind="ExternalInput")
    din["giota"] = nc.dram_tensor("giota", [128, G], F32, kind="ExternalInput")
    din["invc"] = nc.dram_tensor("invc", [128, 4], F32, kind="ExternalInput")
    out = nc.dram_tensor("out", [G, C], F32, kind="ExternalOutput")

    y_slice = [nc.dram_tensor(f"y_slice{l}", [S, H], BF16) for l in (1, 2)]
    y_full = [
        nc.dram_tensor(f"y_full{l}", [NPAD, H], BF16)
        for l in (1, 2)
    ]
    pp = nc.dram_tensor("pp", [G, H], F32)
    pooled = nc.dram_tensor("pooled", [G, H], F32, addr_space="Shared")

    rg = [list(range(NC))]

    from contextlib import ExitStack
    ctx = ExitStack()
    with tile.TileContext(nc) as tc, ctx:
        cpool = ctx.enter_context(tc.tile_pool(name="consts", bufs=1))
        msgp = ctx.enter_context(tc.tile_pool(name="msg", bufs=8))
        selp = ctx.enter_context(tc.tile_pool(name="sel", bufs=8))
        wrk = ctx.enter_context(tc.tile_pool(name="wrk", bufs=4))
        ps_xt = ctx.enter_context(tc.tile_pool(name="psXT", bufs=2, space="PSUM"))
        ps_h = ctx.enter_context(tc.tile_pool(name="psH", bufs=2, space="PSUM"))
        ps_z = ctx.enter_context(tc.tile_pool(name="psZ", bufs=4, space="PSUM"))

        def load_const(name, shape, dt):
            t = cpool.tile(shape, dt, tag=name)
            nc.sync.dma_start(out=t[:], in_=din[name][:])
            return t

        # replicate the 16-partition index upload to 128 partitions
        gidx_sb = cpool.tile([128, TOK // 16], I16, tag="gidx")
        for k in range(8):
            nc.sync.dma_start(
                out=gidx_sb[16 * k:16 * (k + 1), :], in_=din["gidx16"][:]
            )

        dloc_sb = load_const("dloc", [128, TOK // 128], BF16)
        dinv_sb = load_const("dinv", [128, NB], F32)
        acol_sb = load_const("acol", [128, NB], F32)
        batc_sb = load_const("batchf", [128, NB], F32)
        w_sb = {1: load_const("W1", [H, H], F32), 2: load_const("W2", [H, H], F32)}
        wp_sb = load_const("Wp", [H, C], F32)
        w0r_sb = load_const("W0r", [128, H], F32)
        br_sb = {
            0: load_const("b0r", [128, H], F32),
            1: load_const("b1r", [128, H], F32),
            2: load_const("b2r", [128, H], F32),
        }
        bpr_sb = load_const("bpr", [128, C], F32)
        id_sb = load_const("ident", [128, 128], F32)
        iot_sb = load_const("iotar", [128, 128], BF16)
        gio_sb = load_const("giota", [128, G], F32)
        ivc_sb = load_const("invc", [128, 4], F32)

        x_sb = cpool.tile([128, S], F32, tag="x")

        def xblk(J):
            return x_sb[:, J * 128:(J + 1) * 128]

        # ---- layer 0 ----
        for J in range(NB):
            t0 = wrk.tile([128, H], F32, tag="l0")
            nc.vector.scalar_tensor_tensor(
                out=t0[:],
                in0=w0r_sb[:],
                scalar=acol_sb[:, J:J + 1],
                in1=br_sb[0][:],
                op0=mybir.AluOpType.mult,
                op1=mybir.AluOpType.add,
            )
            nc.scalar.activation(xblk(J), t0[:], mybir.ActivationFunctionType.Relu)

        def phase_a_block(layer, J):
            """y[J] = dinv * (x[J] @ W_layer) -> y_slice[layer]"""
            xt_ps = ps_xt.tile([128, 128], F32, tag="xt")
            nc.tensor.transpose(out=xt_ps[:], in_=xblk(J), identity=id_sb[:])
            xt_sb = wrk.tile([128, 128], F32, tag="xt_sb")
            nc.scalar.copy(xt_sb[:], xt_ps[:])
            h_ps = ps_h.tile([128, H], F32, tag="h")
            nc.tensor.matmul(
                out=h_ps[:], lhsT=xt_sb[:], rhs=w_sb[layer][:], start=True, stop=True
            )
            y_sb = wrk.tile([128, H], BF16, tag="y")
            nc.scalar.mul(y_sb[:], h_ps[:], mul=dinv_sb[:, J:J + 1])
            nc.sync.dma_start(
                out=y_slice[layer - 1][J * 128:(J + 1) * 128, :], in_=y_sb[:]
            )

        def allgather(layer, w):
            r0 = W_SLOT0[w]
            nrow = W_ROWS[w]
            nc.gpsimd.collective_compute(
                "AllGather",
                mybir.AluOpType.bypass,
                replica_groups=rg,
                ins=[y_slice[layer - 1][r0:r0 + nrow, :]],
                outs=[y_full[layer - 1][W_BASE[w]:W_BASE[w] + NC * nrow, :]],
            )

        # ---- layer 1 phase A (window-ordered) ----
        for w in range(NWIN):
            for J in range(W_SLOT0[w] // 128, W_SLOT0[w] // 128 + W_BLKS[w]):
                phase_a_block(1, J)
            allgather(1, w)

        qctr = 0

        def aggregate_group(layer, g, post_block_hook=None):
            """Phase B for group g of `layer`: gathers + sel + matmuls + evict.
            post_block_hook(J) runs after block J's new x is written."""
            nonlocal qctr
            blocks = list(_blocks_of_group(g))
            msg_t, sel_t, gtok0 = {}, {}, {}
            for q in range(NWIN):
                if (g, q) not in gather_by_gq:
                    continue
                tok0, ntok, subs = gather_by_gq[(g, q)]
                nslots = ntok // 128
                mt = msgp.tile([128, nslots * H], BF16, tag="msg")
                wq = y_full[layer - 1][q * WIN:min((q + 1) * WIN, NPAD), :]
                mt3 = mt[:].rearrange("p (s e) -> p s e", e=H)
                for (s0, snt, qu) in subs:
                    so = (s0 - tok0) // 128
                    nc.gpsimd.dma_gather(
                        out_ap=mt3[:, so:so + snt // 128, :],
                        in_ap=wq,
                        idxs_ap=gidx_sb[:, s0 // 16:(s0 + snt) // 16],
                        num_idxs=snt,
                        num_idxs_reg=snt,
                        elem_size=H,
                        queue_num=qu,
                        single_packet=False,
                    )
                    qctr += 1
                st = selp.tile([128, nslots * 128], FP8, tag="sel")
                nc.vector.tensor_tensor(
                    out=st[:].rearrange("p (s e) -> p s e", e=128),
                    in0=dloc_sb[:, tok0 // 128:(tok0 + ntok) // 128, None]
                    .to_broadcast([128, nslots, 128]),
                    in1=iot_sb[:, None, :].to_broadcast([128, nslots, 128]),
                    op=mybir.AluOpType.is_equal,
                )
                msg_t[q], sel_t[q], gtok0[q] = mt, st, tok0

            zp = {J: ps_z.tile([128, H], F32, tag="z", name=f"z{J}") for J in blocks}
            tot = {J: int(cq[J].sum()) for J in blocks}
            done = {J: 0 for J in blocks}
            for q in range(NWIN):
                if q not in msg_t:
                    continue
                for J in blocks:
                    nch = int(cq[J, q])
                    if nch == 0:
                        continue
                    s0 = (seg_tok0[J, q] - gtok0[q]) // 128
                    for i in range(nch):
                        s = s0 + i
                        nc.tensor.matmul(
                            out=zp[J][:],
                            lhsT=sel_t[q][:, (s * 128):(s + 1) * 128],
                            rhs=msg_t[q][:].rearrange("p (s e) -> p s e", e=H)[:, s, :],
                            start=(done[J] == 0),
                            stop=(done[J] == tot[J] - 1),
                        )
                        done[J] += 1
            for J in blocks:
                t1 = wrk.tile([128, H], F32, tag="pc")
                nc.vector.scalar_tensor_tensor(
                    out=t1[:],
                    in0=zp[J][:],
                    scalar=dinv_sb[:, J:J + 1],
                    in1=br_sb[layer][:],
                    op0=mybir.AluOpType.mult,
                    op1=mybir.AluOpType.add,
                )
                nc.scalar.activation(xblk(J), t1[:], mybir.ActivationFunctionType.Relu)
                if post_block_hook is not None:
                    post_block_hook(J)

        # ---- layer 1 phase B, interleaved with layer 2 phase A ----
        for g in range(NGRP):
            aggregate_group(1, g, post_block_hook=lambda J: phase_a_block(2, J))
            for w in range(NWIN):
                if g == list(_groups_of_window(w))[-1]:
                    allgather(2, w)

        # ---- layer 2 phase B, interleaved with pooling ----
        psum_pool = [
            (ps_xt if gb < 2 else ps_h).tile(
                [128, H], F32, tag=("xt" if gb < 2 else "h"), name=f"poolacc{gb}"
            )
            for gb in range(4)
        ]

        def pool_block(J):
            selg = wrk.tile([128, G], F32, tag="selg")
            nc.vector.tensor_tensor(
                out=selg[:],
                in0=batc_sb[:, J:J + 1].to_broadcast([128, G]),
                in1=gio_sb[:],
                op=mybir.AluOpType.is_equal,
            )
            for gb in range(4):
                nc.tensor.matmul(
                    out=psum_pool[gb][:],
                    lhsT=selg[:, gb * 128:(gb + 1) * 128],
                    rhs=xblk(J),
                    start=(J == 0),
                    stop=(J == NB - 1),
                )

        for g in range(NGRP):
            aggregate_group(2, g, post_block_hook=pool_block)

        for gb in range(4):
            t2 = wrk.tile([128, H], F32, tag="ppev")
            nc.scalar.copy(t2[:], psum_pool[gb][:])
            nc.sync.dma_start(out=pp[gb * 128:(gb + 1) * 128, :], in_=t2[:])
        nc.gpsimd.collective_compute(
            "AllReduce",
            mybir.AluOpType.add,
            replica_groups=rg,
            ins=[pp[:]],
            outs=[pooled[:]],
        )

        # ---- head + log_softmax ----
        for gb in range(4):
            pl = wrk.tile([128, H], F32, tag="pl")
            nc.sync.dma_start(out=pl[:], in_=pooled[gb * 128:(gb + 1) * 128, :])
            plm = wrk.tile([128, H], F32, tag="plm")
            nc.scalar.mul(plm[:], pl[:], mul=ivc_sb[:, gb:gb + 1])
            pt_ps = ps_xt.tile([128, 128], F32, tag="xt")
            nc.tensor.transpose(out=pt_ps[:], in_=plm[:], identity=id_sb[:])
            pt_sb = wrk.tile([128, 128], F32, tag="pts")
            nc.scalar.copy(pt_sb[:], pt_ps[:])
            lg_ps = ps_h.tile([128, C], F32, tag="h")
            nc.tensor.matmul(
                out=lg_ps[:], lhsT=pt_sb[:], rhs=wp_sb[:], start=True, stop=True
            )
            tl = wrk.tile([128, C], F32, tag="tl")
            nc.vector.tensor_tensor(
                out=tl[:], in0=lg_ps[:], in1=bpr_sb[:], op=mybir.AluOpType.add
            )
            mx = wrk.tile([128, 1], F32, tag="mx")
            nc.vector.tensor_reduce(
                out=mx[:], in_=tl[:], axis=mybir.AxisListType.X,
                op=mybir.AluOpType.max,
            )
            nmx = wrk.tile([128, 1], F32, tag="nmx")
            nc.vector.tensor_scalar_mul(nmx[:], mx[:], -1.0)
            ex = wrk.tile([128, C], F32, tag="ex")
            ssum = wrk.tile([128, 1], F32, tag="ssum")
            nc.scalar.activation(
                ex[:], tl[:], mybir.ActivationFunctionType.Exp,
                bias=nmx[:, :1], accum_out=ssum[:],
            )
            lns = wrk.tile([128, 1], F32, tag="lns")
            nc.scalar.activation(lns[:], ssum[:], mybir.ActivationFunctionType.Ln)
            ofs = wrk.tile([128, 1], F32, tag="ofs")
            nc.vector.tensor_tensor(
                out=ofs[:], in0=nmx[:], in1=lns[:], op=mybir.AluOpType.subtract
            )
            fin = wrk.tile([128, C], F32, tag="fin")
            nc.vector.tensor_scalar_add(fin[:], tl[:], ofs[:, :1])
            nc.sync.dma_start(out=out[gb * 128:(gb + 1) * 128, :], in_=fin[:])

    nc.compile()
    return nc


_CACHE = {}


def kernel(edge_index, batch, W0, b0, W1, b1, W2, b2, Wp, bp):
    edge_index = np.asarray(edge_index, dtype=np.int32)
    batch = np.asarray(batch, dtype=np.int32)
    W0 = np.asarray(W0, dtype=np.float32)
    b0 = np.asarray(b0, dtype=np.float32)
    W1 = np.asarray(W1, dtype=np.float32)
    b1 = np.asarray(b1, dtype=np.float32)
    W2 = np.asarray(W2, dtype=np.float32)
    b2 = np.asarray(b2, dtype=np.float32)
    Wp = np.asarray(Wp, dtype=np.float32)
    bp = np.asarray(bp, dtype=np.float32)

    key = hash((edge_index.tobytes(), batch.tobytes()))
    if key not in _CACHE:
        sched, percore, invcnt = preprocess(edge_index, batch)
        nc = build_program(sched)
        _CACHE[key] = (sched, percore, invcnt, nc)
    sched, percore, invcnt, nc = _CACHE[key]

    consts = {
        "W1": W1,
        "W2": W2,
        "Wp": Wp,
        "W0r": np.tile(W0.reshape(1, H), (128, 1)),
        "b0r": np.tile(b0.reshape(1, H), (128, 1)),
        "b1r": np.tile(b1.reshape(1, H), (128, 1)),
        "b2r": np.tile(b2.reshape(1, H), (128, 1)),
        "bpr": np.tile(bp.reshape(1, C), (128, 1)),
        "ident": np.eye(128, dtype=np.float32),
        "iotar": np.tile(
            np.arange(128, dtype=np.float32).astype(NP_BF16).reshape(1, 128),
            (128, 1),
        ),
        "giota": np.tile(np.arange(G, dtype=np.float32).reshape(1, G), (128, 1)),
        "invc": invcnt,
    }
    consts = {k: np.ascontiguousarray(v) for k, v in consts.items()}

    in_maps = []
    for c in range(NC):
        m = {
            "gidx16": percore["gidx16"][c],
            "dloc": percore["dloc"][c],
            "dinv": percore["dinv_pj"][c],
            "acol": percore["a_pj"][c],
            "batchf": percore["batc_pj"][c],
        }
        m.update(consts)
        in_maps.append(m)

    import os
    trace = bool(int(os.environ.get("KGCN_TRACE", "0")))
    res = run_bass_kernel_spmd(nc, in_maps, core_ids=list(range(NC)), trace=trace)
    kernel.last_results = res
    return res.results[0]["out"]


# revision 4
# speedup vs baseline: 1.0501x; 1.0501x over previous
"""GCN graph classification on 8 Trainium2 NeuronCores (Bass/Tile) — v2.

Structure (vs v1): AllGather chunks == gather windows (32768 table rows),
group-major/window-inner aggregation with PSUM accumulation across windows,
layer-2 feature matmuls + AllGather interleaved into layer-1 aggregation,
pooling interleaved into layer-2 aggregation, fp8 selection masks, and
16-partition index upload replicated on device.
"""
import sys

sys.path.insert(0, "/opt/trn_rl_repo")

import numpy as np
import ml_dtypes

import concourse.bass as bass
import concourse.bacc as bacc
import concourse.mybir as mybir
import concourse.tile as tile
from concourse.bass_utils import run_bass_kernel_spmd

# problem constants
N = 100000
E = 1600000
G = 512
H = 128
C = 10
NC = 8
NB = 98                # blocks per core
S = NB * 128           # node slots per core = 12544
NPAD = NC * S          # table rows = 100352
WIN = 32768
NWIN = 4
GRP = 4
NGRP = (NB + GRP - 1) // GRP  # 25
# window w covers slots [W_SLOT0[w], +W_ROWS[w]) on each core and table rows
# [W_BASE[w], +8*W_ROWS[w]); AllGather chunk == window.
W_BLKS = [32, 32, 32, 2]
W_ROWS = [b * 128 for b in W_BLKS]          # [4096,4096,4096,256]
W_SLOT0 = [0, 4096, 8192, 12288]
W_BASE = [0, 32768, 65536, 98304]

F32 = mybir.dt.float32
BF16 = mybir.dt.bfloat16
FP8 = mybir.dt.float8e4
I16 = mybir.dt.int16
NP_BF16 = ml_dtypes.bfloat16


def _blocks_of_group(g):
    return range(g * GRP, min((g + 1) * GRP, NB))


def _groups_of_window(w):
    # groups fully contained in window w (blocks 32w..32w+31; window 3: 96,97)
    if w < 3:
        return range(8 * w, 8 * (w + 1))
    return range(24, 25)


def preprocess(edge_index, batch):
    edge_index = np.asarray(edge_index, dtype=np.int64)
    batch = np.asarray(batch, dtype=np.int64)

    loop = np.arange(N, dtype=np.int64)
    src = np.concatenate([edge_index[0], loop])
    dst = np.concatenate([edge_index[1], loop])
    EE = src.shape[0]

    deg = np.bincount(dst, minlength=N).astype(np.float64)
    dinv = np.where(deg > 0, 1.0 / np.sqrt(deg), 0.0)
    csum = np.bincount(dst, weights=dinv[src], minlength=N)
    a = (dinv * csum).astype(np.float32)
    dinv32 = dinv.astype(np.float32)

    # node -> (core, slot): snake deal by descending degree
    order = np.argsort(-deg, kind="stable")
    pos = np.arange(N)
    p16 = pos % 16
    core_r = np.where(p16 < 8, p16, 15 - p16)
    j_r = (pos // 16) * 2 + (p16 >= 8)
    core = np.empty(N, dtype=np.int64)
    jwc = np.empty(N, dtype=np.int64)
    core[order] = core_r
    jwc[order] = j_r
    pas = jwc // NB
    r = jwc % NB
    blk = np.where(pas % 2 == 0, r, NB - 1 - r)
    slot = blk * 128 + pas
    assert pas.max() < 128

    # table row: window-aligned layout
    w_of_slot = np.minimum(slot // 4096, 3)
    rows_w = np.array(W_ROWS)[w_of_slot]
    base_w = np.array(W_BASE)[w_of_slot]
    slot0_w = np.array(W_SLOT0)[w_of_slot]
    tr = base_w + core * rows_w + (slot - slot0_w)
    assert tr.min() >= 0 and tr.max() < NPAD

    # per-slot arrays [NC, 128, NB]
    dinv_sl = np.zeros((NC, S), dtype=np.float32)
    a_sl = np.zeros((NC, S), dtype=np.float32)
    batc_sl = np.full((NC, S), -1.0, dtype=np.float32)
    dinv_sl[core, slot] = dinv32
    a_sl[core, slot] = a
    batc_sl[core, slot] = batch.astype(np.float32)

    def to_pj(x):
        return np.ascontiguousarray(x.reshape(NC, NB, 128).transpose(0, 2, 1))

    dinv_pj = to_pj(dinv_sl)
    a_pj = to_pj(a_sl)
    batc_pj = to_pj(batc_sl)

    # edges
    ecore = core[dst]
    eslot = slot[dst]
    eJ = eslot // 128
    eP = (eslot % 128).astype(np.float32)
    etr = tr[src]
    eq = etr // WIN
    eidx = (etr - eq * WIN).astype(np.int16)

    key = (ecore * NB + eJ) * NWIN + eq
    cnt = np.bincount(key, minlength=NC * NB * NWIN).reshape(NC, NB, NWIN)
    cq = np.ceil(cnt.max(axis=0) / 128).astype(np.int64)  # [NB, NWIN]

    # stream layout: for g: for q: for J in group
    seg_tok0 = np.zeros((NB, NWIN), dtype=np.int64)
    gathers = []  # (g, q, tok0, ntok, [(sub_tok0, sub_ntok, queue), ...])
    qload = [0, 0, 0, 0]
    tok = 0
    for g in range(NGRP):
        for q in range(NWIN):
            t0 = tok
            for J in _blocks_of_group(g):
                seg_tok0[J, q] = tok
                tok += cq[J, q] * 128
            ntok = tok - t0
            if ntok == 0:
                continue
            # split into sub-gathers of <= 1792 tokens; greedy queue balance
            nsub = max(1, int(np.ceil(ntok / 1792)))
            bounds = [t0 + (ntok * i // nsub) // 128 * 128 for i in range(nsub)]
            bounds.append(t0 + ntok)
            subs = []
            for i in range(nsub):
                s0, s1 = bounds[i], bounds[i + 1]
                if s1 <= s0:
                    continue
                qu = int(np.argmin(qload))
                qload[qu] += s1 - s0
                subs.append((s0, s1 - s0, qu))
            gathers.append((g, q, t0, ntok, subs))
    TOK = tok
    assert TOK % 128 == 0

    ordk = np.argsort(key, kind="stable")
    skey = key[ordk]
    first = np.searchsorted(skey, skey)
    rank = np.arange(EE) - first
    p_stream = seg_tok0[eJ[ordk], eq[ordk]] + rank

    gidx = np.zeros((NC, TOK), dtype=np.int16)
    dloc = np.full((NC, TOK), -1.0, dtype=np.float32)
    gidx[ecore[ordk], p_stream] = eidx[ordk]
    dloc[ecore[ordk], p_stream] = eP[ordk]

    # device layouts: gidx uploaded as 16 partitions (replicated on device)
    g16 = np.ascontiguousarray(
        gidx.reshape(NC, TOK // 16, 16).transpose(0, 2, 1)
    )  # [NC, 16, TOK//16]
    dloc_dev = np.ascontiguousarray(
        dloc.reshape(NC, TOK // 128, 128).transpose(0, 2, 1)
    ).astype(NP_BF16)

    cntg = np.bincount(batch, minlength=G).astype(np.float32)
    invcnt = (1.0 / np.maximum(cntg, 1.0)).reshape(4, 128).T.copy()

    sched = {"cq": cq, "seg_tok0": seg_tok0, "gathers": gathers, "TOK": TOK}
    percore = {
        "gidx16": g16,
        "dloc": dloc_dev,
        "dinv_pj": dinv_pj,
        "a_pj": a_pj,
        "batc_pj": batc_pj,
    }
    return sched, percore, invcnt


def build_program(sched):
    cq = sched["cq"]
    seg_tok0 = sched["seg_tok0"]
    gathers = sched["gathers"]
    TOK = sched["TOK"]
    gather_by_gq = {(g, q): (t0, nt, subs) for (g, q, t0, nt, subs) in gathers}

    nc = bacc.Bacc(
        "TRN2",
        target_bir_lowering=False,
        debug=False,
        num_devices=NC,
        num_swdge_queues=4,
    )

    din = {}
    din["gidx16"] = nc.dram_tensor("gidx16", [16, TOK // 16], I16, kind="ExternalInput")
    din["dloc"] = nc.dram_tensor("dloc", [128, TOK // 128], BF16, kind="ExternalInput")
    din["dinv"] = nc.dram_tensor("dinv", [128, NB], F32, kind="ExternalInput")
    din["acol"] = nc.dram_tensor("acol", [128, NB], F32, kind="ExternalInput")
    din["batchf"] = nc.dram_tensor("batchf", [128, NB], F32, kind="ExternalInput")
    din["W1"] = nc.dram_tensor("W1", [H, H], F32, kind="ExternalInput")
    din["W2"] = nc.dram_tensor("W2", [H, H], F32, kind="ExternalInput")
    din["Wp"] = nc.dram_tensor("Wp", [H, C], F32, kind="ExternalInput")
    din["W0r"] = nc.dram_tensor("W0r", [128, H], F32, kind="ExternalInput")
    din["b0r"] = nc.dram_tensor("b0r", [128, H], F32, kind="ExternalInput")
    din["b1r"] = nc.dram_tensor("b1r", [128, H], F32, kind="ExternalInput")
    din["b2r"] = nc.dram_tensor("b2r", [128, H], F32, kind="ExternalInput")
    din["bpr"] = nc.dram_tensor("bpr", [128, C], F32, kind="ExternalInput")
    din["ident"] = nc.dram_tensor("ident", [128, 128], F32, kind="ExternalInput")
    din["iotar"] = nc.dram_tensor("iotar", [128, 128], BF16, kind="ExternalInput")
    din["giota"] = nc.dram_tensor("giota", [128, G], F32, kind="ExternalInput")
    din["invc"] = nc.dram_tensor("invc", [128, 4], F32, kind="ExternalInput")
    out = nc.dram_tensor("out", [G, C], F32, kind="ExternalOutput")

    y_slice = [nc.dram_tensor(f"y_slice{l}", [S, H], BF16) for l in (1, 2)]
    y_full = [
        nc.dram_tensor(f"y_full{l}", [NPAD, H], BF16)
        for l in (1, 2)
    ]
    pp = nc.dram_tensor("pp", [G, H], F32)
    pooled = nc.dram_tensor("pooled", [G, H], F32, addr_space="Shared")

    rg = [list(range(NC))]

    from contextlib import ExitStack
    ctx = ExitStack()
    with tile.TileContext(nc) as tc, ctx:
        cpool = ctx.enter_context(tc.tile_pool(name="consts", bufs=1))
        msgp = ctx.enter_context(tc.tile_pool(name="msg", bufs=8))
        selp = ctx.enter_context(tc.tile_pool(name="sel", bufs=8))
        wrk = ctx.enter_context(tc.tile_pool(name="wrk", bufs=4))
        ps_xt = ctx.enter_context(tc.tile_pool(name="psXT", bufs=2, space="PSUM"))
        ps_h = ctx.enter_context(tc.tile_pool(name="psH", bufs=2, space="PSUM"))
        ps_z = ctx.enter_context(tc.tile_pool(name="psZ", bufs=4, space="PSUM"))

        def load_const(name, shape, dt):
            t = cpool.tile(shape, dt, tag=name)
            nc.sync.dma_start(out=t[:], in_=din[name][:])
            return t

        # replicate the 16-partition index upload to 128 partitions
        gidx_sb = cpool.tile([128, TOK // 16], I16, tag="gidx")
        for k in range(8):
            nc.sync.dma_start(
                out=gidx_sb[16 * k:16 * (k + 1), :], in_=din["gidx16"][:]
            )

        dloc_sb = load_const("dloc", [128, TOK // 128], BF16)
        dinv_sb = load_const("dinv", [128, NB], F32)
        acol_sb = load_const("acol", [128, NB], F32)
        batc_sb = load_const("batchf", [128, NB], F32)
        w_sb = {1: load_const("W1", [H, H], F32), 2: load_const("W2", [H, H], F32)}
        wp_sb = load_const("Wp", [H, C], F32)
        w0r_sb = load_const("W0r", [128, H], F32)
        br_sb = {
            0: load_const("b0r", [128, H], F32),
            1: load_const("b1r", [128, H], F32),
            2: load_const("b2r", [128, H], F32),
        }
        bpr_sb = load_const("bpr", [128, C], F32)
        id_sb = load_const("ident", [128, 128], F32)
        iot_sb = load_const("iotar", [128, 128], BF16)
        gio_sb = load_const("giota", [128, G], F32)
        ivc_sb = load_const("invc", [128, 4], F32)

        x_sb = cpool.tile([128, S], F32, tag="x")

        def xblk(J):
            return x_sb[:, J * 128:(J + 1) * 128]

        # ---- layer 0 ----
        for J in range(NB):
            t0 = wrk.tile([128, H], F32, tag="l0")
            nc.vector.scalar_tensor_tensor(
                out=t0[:],
                in0=w0r_sb[:],
                scalar=acol_sb[:, J:J + 1],
                in1=br_sb[0][:],
                op0=mybir.AluOpType.mult,
                op1=mybir.AluOpType.add,
            )
            nc.scalar.activation(xblk(J), t0[:], mybir.ActivationFunctionType.Relu)

        def phase_a_block(layer, J):
            """y[J] = dinv * (x[J] @ W_layer) -> y_slice[layer]"""
            xt_ps = ps_xt.tile([128, 128], F32, tag="xt")
            nc.tensor.transpose(out=xt_ps[:], in_=xblk(J), identity=id_sb[:])
            xt_sb = wrk.tile([128, 128], F32, tag="xt_sb")
            nc.scalar.copy(xt_sb[:], xt_ps[:])
            h_ps = ps_h.tile([128, H], F32, tag="h")
            nc.tensor.matmul(
                out=h_ps[:], lhsT=xt_sb[:], rhs=w_sb[layer][:], start=True, stop=True
            )
            y_sb = wrk.tile([128, H], BF16, tag="y")
            nc.scalar.mul(y_sb[:], h_ps[:], mul=dinv_sb[:, J:J + 1])
            nc.sync.dma_start(
                out=y_slice[layer - 1][J * 128:(J + 1) * 128, :], in_=y_sb[:]
            )

        def allgather(layer, w):
            r0 = W_SLOT0[w]
            nrow = W_ROWS[w]
            nc.gpsimd.collective_compute(
                "AllGather",
                mybir.AluOpType.bypass,
                replica_groups=rg,
                ins=[y_slice[layer - 1][r0:r0 + nrow, :]],
                outs=[y_full[layer - 1][W_BASE[w]:W_BASE[w] + NC * nrow, :]],
            )

        # ---- layer 1 phase A (window-ordered) ----
        for w in range(NWIN):
            for J in range(W_SLOT0[w] // 128, W_SLOT0[w] // 128 + W_BLKS[w]):
                phase_a_block(1, J)
            allgather(1, w)

        qctr = 0

        def aggregate_group(layer, g, post_block_hook=None):
            """Phase B for group g of `layer`: gathers + sel + matmuls + evict.
            post_block_hook(J) runs after block J's new x is written."""
            nonlocal qctr
            blocks = list(_blocks_of_group(g))
            msg_t, sel_t, gtok0 = {}, {}, {}
            for q in range(NWIN):
                if (g, q) not in gather_by_gq:
                    continue
                tok0, ntok, subs = gather_by_gq[(g, q)]
                nslots = ntok // 128
                mt = msgp.tile([128, nslots * H], BF16, tag="msg")
                wq = y_full[layer - 1][q * WIN:min((q + 1) * WIN, NPAD), :]
                mt3 = mt[:].rearrange("p (s e) -> p s e", e=H)
                for (s0, snt, qu) in subs:
                    so = (s0 - tok0) // 128
                    nc.gpsimd.dma_gather(
                        out_ap=mt3[:, so:so + snt // 128, :],
                        in_ap=wq,
                        idxs_ap=gidx_sb[:, s0 // 16:(s0 + snt) // 16],
                        num_idxs=snt,
                        num_idxs_reg=snt,
                        elem_size=H,
                        queue_num=qu,
                        single_packet=False,
                    )
                    qctr += 1
                st = selp.tile([128, nslots * 128], FP8, tag="sel")
                nc.vector.tensor_tensor(
                    out=st[:].rearrange("p (s e) -> p s e", e=128),
                    in0=dloc_sb[:, tok0 // 128:(tok0 + ntok) // 128, None]
                    .to_broadcast([128, nslots, 128]),
                    in1=iot_sb[:, None, :].to_broadcast([128, nslots, 128]),
                    op=mybir.AluOpType.is_equal,
                )
                msg_t[q], sel_t[q], gtok0[q] = mt, st, tok0

            zp = {J: ps_z.tile([128, H], F32, tag="z", name=f"z{J}") for J in blocks}
            tot = {J: int(cq[J].sum()) for J in blocks}
            done = {J: 0 for J in blocks}
            for q in range(NWIN):
                if q not in msg_t:
                    continue
                for J in blocks:
                    nch = int(cq[J, q])
                    if nch == 0:
                        continue
                    s0 = (seg_tok0[J, q] - gtok0[q]) // 128
                    for i in range(nch):
                        s = s0 + i
                        nc.tensor.matmul(
                            out=zp[J][:],
                            lhsT=sel_t[q][:, (s * 128):(s + 1) * 128],
                            rhs=msg_t[q][:].rearrange("p (s e) -> p s e", e=H)[:, s, :],
                            start=(done[J] == 0),
                            stop=(done[J] == tot[J] - 1),
                        )
                        done[J] += 1
            for J in blocks:
                t1 = wrk.tile([128, H], F32, tag="pc")
                nc.vector.scalar_tensor_tensor(
                    out=t1[:],
                    in0=zp[J][:],
                    scalar=dinv_sb[:, J:J + 1],
                    in1=br_sb[layer][:],
                    op0=mybir.AluOpType.mult,
                    op1=mybir.AluOpType.add,
                )
                nc.scalar.activation(xblk(J), t1[:], mybir.ActivationFunctionType.Relu)
                if post_block_hook is not None:
                    post_block_hook(J)

        # ---- layer 1 phase B, interleaved with layer 2 phase A ----
        for g in range(NGRP):
            aggregate_group(1, g, post_block_hook=lambda J: phase_a_block(2, J))
            for w in range(NWIN):
                if g == list(_groups_of_window(w))[-1]:
                    allgather(2, w)

        # ---- layer 2 phase B, interleaved with pooling ----
        psum_pool = [
            (ps_xt if gb < 2 else ps_h).tile(
                [128, H], F32, tag=("xt" if gb < 2 else "h"), name=f"poolacc{gb}"
            )
            for gb in range(4)
        ]

        def pool_block(J):
            selg = wrk.tile([128, G], F32, tag="selg")
            nc.vector.tensor_tensor(
                out=selg[:],
                in0=batc_sb[:, J:J + 1].to_broadcast([128, G]),
                in1=gio_sb[:],
                op=mybir.AluOpType.is_equal,
            )
            for gb in range(4):
                nc.tensor.matmul(
                    out=psum_pool[gb][:],
                    lhsT=selg[:, gb * 128:(gb + 1) * 128],
                    rhs=xblk(J),
                    start=(J == 0),
                    stop=(J == NB - 1),
                )

        for g in range(NGRP):
            aggregate_group(2, g, post_block_hook=pool_block)

        for gb in range(4):
            t2 = wrk.tile([128, H], F32, tag="ppev")
            nc.scalar.copy(t2[:], psum_pool[gb][:])
            nc.sync.dma_start(out=pp[gb * 128:(gb + 1) * 128, :], in_=t2[:])
        nc.gpsimd.collective_compute(
            "AllReduce",
            mybir.AluOpType.add,
            replica_groups=rg,
            ins=[pp[:]],
            outs=[pooled[:]],
        )

        # ---- head + log_softmax ----
        for gb in range(4):
            pl = wrk.tile([128, H], F32, tag="pl")
            nc.sync.dma_start(out=pl[:], in_=pooled[gb * 128:(gb + 1) * 128, :])
            plm = wrk.tile([128, H], F32, tag="plm")
            nc.scalar.mul(plm[:], pl[:], mul=ivc_sb[:, gb:gb + 1])
            pt_ps = ps_xt.tile([128, 128], F32, tag="xt")
            nc.tensor.transpose(out=pt_ps[:], in_=plm[:], identity=id_sb[:])
            pt_sb = wrk.tile([128, 128], F32, tag="pts")
            nc.scalar.copy(pt_sb[:], pt_ps[:])
            lg_ps = ps_h.tile([128, C], F32, tag="h")
            nc.tensor.matmul(
                out=lg_ps[:], lhsT=pt_sb[:], rhs=wp_sb[:], start=True, stop=True
            )
            tl = wrk.tile([128, C], F32, tag="tl")
            nc.vector.tensor_tensor(
                out=tl[:], in0=lg_ps[:], in1=bpr_sb[:], op=mybir.AluOpType.add
            )
            mx = wrk.tile([128, 1], F32, tag="mx")
            nc.vector.tensor_reduce(
                out=mx[:], in_=tl[:], axis=mybir.AxisListType.X,
                op=mybir.AluOpType.max,
            )
            nmx = wrk.tile([128, 1], F32, tag="nmx")
            nc.vector.tensor_scalar_mul(nmx[:], mx[:], -1.0)
            ex = wrk.tile([128, C], F32, tag="ex")
            ssum = wrk.tile([128, 1], F32, tag="ssum")
            nc.scalar.activation(
                ex[:], tl[:], mybir.ActivationFunctionType.Exp,
                bias=nmx[:, :1], accum_out=ssum[:],
            )
            lns = wrk.tile([128, 1], F32, tag="lns")
            nc.scalar.activation(lns[:], ssum[:], mybir.ActivationFunctionType.Ln)
            ofs = wrk.tile([128, 1], F32, tag="ofs")
            nc.vector.tensor_tensor(
                out=ofs[:], in0=nmx[:], in1=lns[:], op=mybir.AluOpType.subtract
            )
            fin = wrk.tile([128, C], F32, tag="fin")
            nc.vector.tensor_scalar_add(fin[:], tl[:], ofs[:, :1])
            nc.sync.dma_start(out=out[gb * 128:(gb + 1) * 128, :], in_=fin[:])

    nc.compile()
    return nc


_CACHE = {}


def kernel(edge_index, batch, W0, b0, W1, b1, W2, b2, Wp, bp):
    edge_index = np.asarray(edge_index, dtype=np.int32)
    batch = np.asarray(batch, dtype=np.int32)
    W0 = np.asarray(W0, dtype=np.float32)
    b0 = np.asarray(b0, dtype=np.float32)
    W1 = np.asarray(W1, dtype=np.float32)
    b1 = np.asarray(b1, dtype=np.float32)
    W2 = np.asarray(W2, dtype=np.float32)
    b2 = np.asarray(b2, dtype=np.float32)
    Wp = np.asarray(Wp, dtype=np.float32)
    bp = np.asarray(bp, dtype=np.float32)

    key = hash((edge_index.tobytes(), batch.tobytes()))
    if key not in _CACHE:
        sched, percore, invcnt = preprocess(edge_index, batch)
        nc = build_program(sched)
        _CACHE[key] = (sched, percore, invcnt, nc)
    sched, percore, invcnt, nc = _CACHE[key]

    consts = {
        "W1": W1,
        "W2": W2,
        "Wp": Wp,
        "W0r": np.tile(W0.reshape(1, H), (128, 1)),
        "b0r": np.tile(b0.reshape(1, H), (128, 1)),
        "b1r": np.tile(b1.reshape(1, H), (128, 1)),
        "b2r": np.tile(b2.reshape(1, H), (128, 1)),
        "bpr": np.tile(bp.reshape(1, C), (128, 1)),
        "ident": np.eye(128, dtype=np.float32),
        "iotar": np.tile(
            np.arange(128, dtype=np.float32).astype(NP_BF16).reshape(1, 128),
            (128, 1),
        ),
        "giota": np.tile(np.arange(G, dtype=np.float32).reshape(1, G), (128, 1)),
        "invc": invcnt,
    }
    consts = {k: np.ascontiguousarray(v) for k, v in consts.items()}

    in_maps = []
    for c in range(NC):
        m = {
            "gidx16": percore["gidx16"][c],
            "dloc": percore["dloc"][c],
            "dinv": percore["dinv_pj"][c],
            "acol": percore["a_pj"][c],
            "batchf": percore["batc_pj"][c],
        }
        m.update(consts)
        in_maps.append(m)

    import os
    trace = bool(int(os.environ.get("KGCN_TRACE", "0")))
    res = run_bass_kernel_spmd(nc, in_maps, core_ids=list(range(NC)), trace=trace)
    kernel.last_results = res
    return res.results[0]["out"]
